# revision 57
# baseline (speedup 1.0000x reference)
"""MemoryGate kernel for Trainium2 (8 NeuronCores, SPMD), fp8 DoubleRow.

Math (per batch b):
    mp   = memory[b] @ W_mem.T                      [M, D]
    S    = hidden[b] @ mp.T / sqrt(D)               [N, M]
    A    = softmax(S, axis=-1)
    ctx  = A @ mp                                   [N, D]
    gate = sigmoid(hidden @ Wg_h.T + ctx @ Wg_c.T + b_gate)
    out  = rmsnorm(hidden + gate * ctx) * norm_w

Sharding: 8 cores = 4 batches x 2 N-halves. Each core computes mp for its
batch (duplicated across the pair) and processes N/2 = 2048 rows.

All big matmuls run fp8(e4m3) with perf_mode=DoubleRow: operands laid out
[128, ktiles, X] and sliced [:, 2t:2t+2, :] so each MM contracts K=256.
PSUM accumulates fp32. Softmax normalization is deferred: exp() writes
unnormalized fp8 attn weights (bias=-1 keeps the range within e4m3), the
fp32 row sums come from the activation accumulator, and 1/sum is applied
in the ctx PSUM drain (so ctx, ctxT and the gate see normalized values).

Layout strategy (per core, all [partition, free]):
    hT8     [D, BN] fp8 (host pre-transposed)  -> lhsT for scores / gate-G1
    memT8   [E, M], WmT8 [E, D] fp8            -> stage A operands
    mpT8    [D, M] fp8 (unscaled; 1/sqrt(D) folded into the exp scale)
    mp8     [M, D] fp8                          -> rhs for ctx
    scores PSUM [n-part, m-free]; exp -> attn8 fp8 (unnormalized)
    attn8 transposed on PE (fp8 128x128 blocks) -> attnT8 lhsT for ctx
    ctx drained with *1/rowsum -> ctx16 bf16; PE-transposed -> ctxT8 fp8
    gate rhs (Wgh/Wgc fp8) streamed from DRAM in [128,2,512] k-pair chunks
    b_gate added via a K=1 bf16 matmul into the same PSUM accumulation
    rmsnorm along free dim in [n-part, d-free] fp32 layout
"""

import math
import os
import sys

for _p in ("/opt/trn_rl_repo", "/root/.axon_site/_ro/trn_rl_repo"):
    if os.path.isdir(_p) and _p not in sys.path:
        sys.path.append(_p)

import numpy as np

P = 128


def build_program(BN=2048, M=2048, D=2048, E=1024, NB=512, FC=512,
                  stop_after=None):
    """Build the per-core Bass program. All shapes must divide evenly.

    stop_after: debug aid — truncate the program after a named phase
    ("A", "scores", "attnT", "ctx", "ctxT", "gate"); None = full kernel.
    """
    import concourse.tile as tile
    from concourse import bacc, mybir

    f32 = mybir.dt.float32
    bf16 = mybir.dt.bfloat16
    fp8 = mybir.dt.float8e4
    AF = mybir.ActivationFunctionType
    AX = mybir.AxisListType
    ALU = mybir.AluOpType
    DR = mybir.MatmulPerfMode.DoubleRow

    kE, kD, mT, nT = E // P, D // P, M // P, NB // P
    NBLK = BN // NB
    mFC, dFC = M // FC, D // FC
    kEh, kDh, mTh = kE // 2, kD // 2, mT // 2
    SCALE = 1.0 / math.sqrt(D)
    EXPB = -2.5          # exp(s/sqrt(D) - 2.5): keeps fp8 attn well under 240
                         # (empirical max s ~ 6.6; e^{6.6-2.5} ~ 60, Inf at 7.98)
    EPS = 1e-6

    nc = bacc.Bacc("TRN2", target_bir_lowering=False, debug=False)

    hT = nc.dram_tensor("hiddenT", [D, BN], fp8, kind="ExternalInput")
    hid = nc.dram_tensor("hidden", [BN, D], bf16, kind="ExternalInput")
    memT = nc.dram_tensor("memT", [E, M], fp8, kind="ExternalInput")
    WmT = nc.dram_tensor("WmT", [E, D], fp8, kind="ExternalInput")
    WghT = nc.dram_tensor("WghT", [D, D], fp8, kind="ExternalInput")
    WgcT = nc.dram_tensor("WgcT", [D, D], fp8, kind="ExternalInput")
    bg = nc.dram_tensor("b_gate", [1, D], bf16, kind="ExternalInput")
    nw = nc.dram_tensor("norm_w", [1, D], f32, kind="ExternalInput")
    id8d = nc.dram_tensor("ident8", [P, P], fp8, kind="ExternalInput")
    id16d = nc.dram_tensor("ident16", [P, P], bf16, kind="ExternalInput")
    oned = nc.dram_tensor("ones", [1, P], bf16, kind="ExternalInput")
    out = nc.dram_tensor("out", [BN, D], f32, kind="ExternalOutput")

    with tile.TileContext(nc) as tc:
        with tc.tile_pool(name="const", bufs=1) as const:
            ident16 = const.tile([P, P], bf16, tag="id16", name="id16_sb")
            ones8_sb = const.tile([P, 2, P], fp8, tag="ones8", name="ones8_sb")
            nc.vector.memset(ones8_sb, 1.0)
            ones_sb = const.tile([1, P], bf16, tag="ones", name="ones_sb")
            bias_sb = const.tile([1, D], bf16, tag="bias", name="bias_sb")
            nw_sb = const.tile([P, D], f32, tag="nw", name="nw_sb")
            eps_t = const.tile([P, 1], f32, tag="eps", name="eps_sb")
            nc.vector.memset(eps_t, EPS)
            expb_t = const.tile([P, 1], f32, tag="expb", name="expb_sb")
            nc.vector.memset(expb_t, EXPB)

            # mpT8 + mp8 stay resident in SBUF for the whole kernel (8 MiB)
            hold_cm = tc.tile_pool(name="hold", bufs=1)
            hold = hold_cm.__enter__()
            mpT8 = hold.tile([P, kD, M], fp8, tag="mpT", name="mpT_sb")
            mp8 = hold.tile([P, mT, D], fp8, tag="mp", name="mp_sb")

            # ---------------- Stage A: mpT8 and mp8 (both unscaled) --------
            with (
                tc.tile_pool(name="a_in", bufs=1) as a_in,
                tc.tile_pool(name="a_ps", bufs=8, space="PSUM") as a_ps,
            ):
                # per-k-pair tiles so the first matmul starts after the
                # first pair lands instead of after all of E
                memT_p = [a_in.tile([P, 2, M], fp8, tag=f"memT{t}",
                                    name=f"memT_sb{t}") for t in range(kEh)]
                WmT_p = [a_in.tile([P, 2, D], fp8, tag=f"WmT{t}",
                                   name=f"WmT_sb{t}") for t in range(kEh)]
                for t in range(kEh):
                    for s in range(2):
                        k = 2 * t + s
                        nc.sync.dma_start(memT_p[t][:, s, :], memT[k * P:(k + 1) * P, :])
                        nc.scalar.dma_start(WmT_p[t][:, s, :], WmT[k * P:(k + 1) * P, :])
                # consts are not needed until much later; issue their DMAs
                # behind the stage-A operands
                nc.sync.dma_start(ident16, id16d[:])
                nc.sync.dma_start(ones_sb, oned[:])
                nc.sync.dma_start(bias_sb, bg[:])
                nc.gpsimd.dma_start(nw_sb, nw[:].partition_broadcast(P))
                # PE warmup: ~4us of junk matmuls on memset data while the
                # stage-A DMAs land, so the HAM un-throttles (1.2->2.4GHz)
                # before the first real matmul issues
                warm_ps = a_ps.tile([P, P], f32, tag="ps", name="warm_ps")
                for w in range(48):
                    nc.tensor.matmul(warm_ps, ones8_sb[:, 0, :], ones8_sb[:, 0, :],
                                     start=True, stop=True)
                # A1: mpT[d, m] = sum_e WmT[e, d] * memT[e, m]
                # For the first 16 groups the k-pair loop runs OUTER over a
                # batch of 8 open PSUM groups, so the PE starts on k-pair 0
                # while the later pairs are still being DMA'd in (the kernel
                # head is input-bandwidth-bound); the rest run k-inner.
                for half in range(2):
                    a1ps = {}
                    for g in range(8):
                        dp, mc = (half * 8 + g) // mFC, (half * 8 + g) % mFC
                        a1ps[g] = a_ps.tile([P, FC], f32, tag="ps",
                                            name=f"a1ps{dp}_{mc}")
                    for t in range(kEh):
                        for g in range(8):
                            dp, mc = (half * 8 + g) // mFC, (half * 8 + g) % mFC
                            nc.tensor.matmul(
                                a1ps[g],
                                WmT_p[t][:, :, dp * P:(dp + 1) * P],
                                memT_p[t][:, :, mc * FC:(mc + 1) * FC],
                                start=(t == 0), stop=(t == kEh - 1),
                                perf_mode=DR,
                            )
                    for g in range(8):
                        dp, mc = (half * 8 + g) // mFC, (half * 8 + g) % mFC
                        nc.scalar.copy(mpT8[:, dp, mc * FC:(mc + 1) * FC], a1ps[g])
                for dp in range(4, kD):
                    for mc in range(mFC):
                        ps = a_ps.tile([P, FC], f32, tag="ps", name=f"a1ps{dp}_{mc}")
                        for t in range(kEh):
                            nc.tensor.matmul(
                                ps,
                                WmT_p[t][:, :, dp * P:(dp + 1) * P],
                                memT_p[t][:, :, mc * FC:(mc + 1) * FC],
                                start=(t == 0), stop=(t == kEh - 1),
                                perf_mode=DR,
                            )
                        nc.scalar.copy(mpT8[:, dp, mc * FC:(mc + 1) * FC], ps)
                # A2: mp[m, d] = sum_e memT[e, m] * WmT[e, d]
                for mp_ in range(mT):
                    for dc in range(dFC):
                        ps = a_ps.tile([P, FC], f32, tag="ps", name=f"a2ps{mp_}_{dc}")
                        for t in range(kEh):
                            nc.tensor.matmul(
                                ps,
                                memT_p[t][:, :, mp_ * P:(mp_ + 1) * P],
                                WmT_p[t][:, :, dc * FC:(dc + 1) * FC],
                                start=(t == 0), stop=(t == kEh - 1),
                                perf_mode=DR,
                            )
                        nc.vector.tensor_copy(mp8[:, mp_, dc * FC:(dc + 1) * FC], ps)

            # ---------------- Stage B: per N-block pipeline -----------------
            with (
                tc.tile_pool(name="b_big", bufs=1) as bb,
                tc.tile_pool(name="b_strm", bufs=12) as strm,
                tc.tile_pool(name="b_sm", bufs=2) as sm,
                tc.tile_pool(name="b_ps", bufs=6, space="PSUM") as bps,
            ):
                def emit_fused_mul(fblk, fgate, fctxt):
                    # fused = hidden + gate*ctx (DVE), chunked so it runs
                    # under the same block's gate matmuls as the sigmoids
                    # complete. Returns the per-i fo tiles for the deferred
                    # rmsnorm part.
                    fn0 = fblk * NB
                    fos = []
                    for i in range(nT):
                        hid_t = strm.tile([P, D], bf16, tag="hid", bufs=2,
                                          name=f"hid{fblk}_{i}")
                        nc.sync.dma_start(hid_t, hid[fn0 + i * P: fn0 + (i + 1) * P, :])
                        fo = strm.tile([P, D], bf16, tag="fo", bufs=4, name=f"fo{fblk}_{i}")
                        for dc in range(dFC):
                            cs = slice(dc * FC, (dc + 1) * FC)
                            nc.vector.tensor_mul(fo[:, cs], fgate[:, i, cs], fctxt[:, i, cs])
                            nc.vector.tensor_add(fo[:, cs], fo[:, cs], hid_t[:, cs])
                        fos.append(fo)
                    return fos

                def emit_fused_norm(fblk, fos, split_finals=False):
                    # rmsnorm + norm_w scale + store. Emitted AFTER the next
                    # block's scores, so its ACT work (squares) queues behind
                    # the exp PSUM-drains the PE is waiting on, not ahead.
                    fn0 = fblk * NB
                    for i in range(nT):
                        fo = fos[i]
                        sq = strm.tile([P, FC], f32, tag="sq", bufs=2, name=f"sq{fblk}_{i}")
                        ssqp = sm.tile([P, dFC], f32, tag="ssq", name=f"ssqp{fblk}_{i}")
                        for dc in range(dFC):
                            cs = slice(dc * FC, (dc + 1) * FC)
                            nc.scalar.activation(sq, fo[:, cs], AF.Square,
                                                 accum_out=ssqp[:, dc:dc + 1])
                        ssq = sm.tile([P, 1], f32, tag="ssq", name=f"ssq{fblk}_{i}")
                        nc.vector.reduce_sum(out=ssq, in_=ssqp, axis=AX.X)
                        rstd = sm.tile([P, 1], f32, tag="rstd", name=f"rstd{fblk}_{i}")
                        nc.scalar.activation(rstd, ssq, AF.Sqrt, bias=eps_t, scale=1.0 / D)
                        nc.vector.reciprocal(rstd, rstd)
                        for dc in range(dFC):
                            cs = slice(dc * FC, (dc + 1) * FC)
                            ov = strm.tile([P, FC], f32, tag="ov", bufs=3,
                                           name=f"ov{fblk}_{i}_{dc}")
                            if split_finals:
                                # tail: split across ACT+DVE so the two
                                # engines drain the last block in parallel
                                nc.scalar.mul(ov, fo[:, cs], rstd)
                                nc.vector.tensor_mul(ov, ov, nw_sb[:, cs])
                            else:
                                # out = (fo * rstd) * norm_w in one DVE pass
                                nc.vector.scalar_tensor_tensor(
                                    ov, fo[:, cs], rstd, nw_sb[:, cs],
                                    op0=ALU.mult, op1=ALU.mult)
                            nc.gpsimd.dma_start(out[fn0 + i * P: fn0 + (i + 1) * P, cs], ov)

                pending_fused = None
                for blk in range(NBLK):
                    n0 = blk * NB
                    hT_sb = bb.tile([P, kD, NB], fp8, tag="hT", bufs=2,
                                    name=f"hT{blk}")
                    for k in range(kD):
                        nc.gpsimd.dma_start(hT_sb[:, k, :], hT[k * P:(k + 1) * P, n0:n0 + NB])

                    if stop_after == "A":
                        if blk == 0:
                            for mt in range(mT):
                                cp = strm.tile([P, D], f32, tag="dbg", bufs=2,
                                               name=f"dbgA{mt}")
                                nc.vector.tensor_copy(cp, mp8[:, mt, :])
                                nc.sync.dma_start(out[mt * P:(mt + 1) * P, :], cp)
                        continue
                    # scores computed TRANSPOSED: S^T[m, n] = mpT8.T @ hT8,
                    # so exp() writes fp8 attnT directly (no PE transposes,
                    # no PSUM->SBUF casts). Softmax denominators = column
                    # sums of the quantized attnT via an all-ones DoubleRow
                    # matmul; a 4x 128x128 PE transpose of the (replicated)
                    # denominator rows yields them in [n,1] layout for the
                    # deferred normalization in the ctx drain.
                    attnT = bb.tile([P, mT, NB], fp8, tag="attnT", name=f"attnT{blk}")
                    rs = sm.tile([P, nT], f32, tag="rs", name=f"rs{blk}")
                    dn_ps = bps.tile([P, NB], f32, tag="dn", bufs=2, name=f"dn{blk}")
                    for mt in range(mT):
                        ps_s = bps.tile([P, NB], f32, tag="ps", name=f"st{blk}_{mt}")
                        for t in range(kDh):
                            nc.tensor.matmul(
                                ps_s,
                                mpT8[:, 2 * t:2 * t + 2, mt * P:(mt + 1) * P],
                                hT_sb[:, 2 * t:2 * t + 2, :],
                                start=(t == 0), stop=(t == kDh - 1),
                                perf_mode=DR,
                            )
                        nc.scalar.activation(attnT[:, mt, :], ps_s, AF.Exp,
                                             scale=SCALE, bias=expb_t)
                        if mt % 2 == 1:
                            nc.tensor.matmul(
                                dn_ps, ones8_sb,
                                attnT[:, mt - 1:mt + 1, :],
                                start=(mt == 1), stop=(mt == mT - 1),
                                perf_mode=DR,
                            )
                    dsb = sm.tile([P, NB], bf16, tag="dsb", name=f"dsb{blk}")
                    nc.vector.tensor_copy(dsb, dn_ps)
                    for j in range(nT):
                        tp = bps.tile([P, P], bf16, tag="dn", bufs=2,
                                      name=f"tpd{blk}_{j}")
                        nc.tensor.transpose(tp, dsb[:, j * P:(j + 1) * P], ident16)
                        nc.vector.tensor_copy(rs[:, j:j + 1], tp[:, 0:1])
                    nc.vector.reciprocal(rs, rs)

                    if stop_after == "scores":
                        for mt in range(mT):
                            if blk == 0:
                                cp = strm.tile([P, NB], f32, tag="dbg", bufs=2,
                                               name=f"dbgS{mt}")
                                nc.vector.tensor_copy(cp, attnT[:, mt, :])
                                nc.sync.dma_start(out[mt * P:(mt + 1) * P, 0:NB], cp)
                        continue

                    if stop_after == "attnT":
                        continue
                    if pending_fused is not None:
                        emit_fused_norm(*pending_fused)
                        pending_fused = None
                    # ctx = (attn @ mp) * 1/rowsum  (normalization in the drain)
                    ctxt = bb.tile([P, nT, D], bf16, tag="ctx", bufs=2, name=f"ctx{blk}")
                    for dc in range(dFC):
                        pss = [bps.tile([P, FC], f32, tag="ps", name=f"cx{blk}_{dc}_{i}")
                               for i in range(nT)]
                        for t in range(mTh):
                            for i in range(nT):
                                nc.tensor.matmul(
                                    pss[i],
                                    attnT[:, 2 * t:2 * t + 2, i * P:(i + 1) * P],
                                    mp8[:, 2 * t:2 * t + 2, dc * FC:(dc + 1) * FC],
                                    start=(t == 0), stop=(t == mTh - 1),
                                    perf_mode=DR,
                                )
                        for i in range(nT):
                            nc.scalar.mul(
                                ctxt[:, i, dc * FC:(dc + 1) * FC], pss[i], rs[:, i:i + 1])

                    if stop_after == "ctx":
                        for i in range(nT):
                            cp = strm.tile([P, D], f32, tag="dbg", bufs=2,
                                           name=f"dbgC{blk}_{i}")
                            nc.vector.tensor_copy(cp, ctxt[:, i, :])
                            nc.sync.dma_start(out[n0 + i * P:n0 + (i + 1) * P, :], cp)
                        continue
                    # gate = sigmoid(hidden @ WghT + ctx @ WgcT + b_gate)
                    # ctx->ctxT transposes run between G1(dc=0) and G2(dc=0):
                    # the G1 matmuls depend only on hT/Wgh, so the PE chews
                    # through them while the ctx PSUM drains finish
                    ctxT = bb.tile([P, kD, NB], fp8, tag="ctxT", name=f"ctxT{blk}")
                    gate = bb.tile([P, nT, D], bf16, tag="gate", name=f"gate{blk}")
                    for dc in range(dFC):
                        pss = [bps.tile([P, FC], f32, tag="ps", name=f"gt{blk}_{dc}_{i}")
                               for i in range(nT)]
                        for t in range(kDh):
                            ch = strm.tile([P, 2, FC], fp8, tag="rhs", name=f"g1ch{blk}_{dc}_{t}")
                            nc.sync.dma_start(
                                ch[:, 0, :], WghT[2 * t * P:(2 * t + 1) * P, dc * FC:(dc + 1) * FC])
                            nc.sync.dma_start(
                                ch[:, 1, :], WghT[(2 * t + 1) * P:(2 * t + 2) * P, dc * FC:(dc + 1) * FC])
                            for i in range(nT):
                                nc.tensor.matmul(
                                    pss[i],
                                    hT_sb[:, 2 * t:2 * t + 2, i * P:(i + 1) * P], ch,
                                    start=(t == 0), stop=False, perf_mode=DR)
                        if dc == 0:
                            for dt_ in range(kD):
                                tp = bps.tile([P, NB], bf16, tag="dn", bufs=2,
                                              name=f"tpc{blk}_{dt_}")
                                for i in range(nT):
                                    nc.tensor.transpose(
                                        tp[:, i * P:(i + 1) * P],
                                        ctxt[:, i, dt_ * P:(dt_ + 1) * P], ident16)
                                nc.vector.tensor_copy(ctxT[:, dt_, :], tp)
                        for t in range(kDh):
                            ch = strm.tile([P, 2, FC], fp8, tag="rhs", name=f"g2ch{blk}_{dc}_{t}")
                            nc.sync.dma_start(
                                ch[:, 0, :], WgcT[2 * t * P:(2 * t + 1) * P, dc * FC:(dc + 1) * FC])
                            nc.sync.dma_start(
                                ch[:, 1, :], WgcT[(2 * t + 1) * P:(2 * t + 2) * P, dc * FC:(dc + 1) * FC])
                            for i in range(nT):
                                nc.tensor.matmul(
                                    pss[i],
                                    ctxT[:, 2 * t:2 * t + 2, i * P:(i + 1) * P], ch,
                                    start=False, stop=False, perf_mode=DR)
                        for i in range(nT):
                            nc.tensor.matmul(
                                pss[i], ones_sb, bias_sb[:, dc * FC:(dc + 1) * FC],
                                start=False, stop=True)
                        for i in range(nT):
                            nc.scalar.activation(
                                gate[:, i, dc * FC:(dc + 1) * FC], pss[i], AF.Sigmoid)

                    if stop_after == "gate":
                        for i in range(nT):
                            cp = strm.tile([P, D], f32, tag="dbg", bufs=2,
                                           name=f"dbgG{blk}_{i}")
                            nc.vector.tensor_copy(cp, gate[:, i, :])
                            nc.sync.dma_start(out[n0 + i * P:n0 + (i + 1) * P, :], cp)
                        continue
                    if blk < NBLK - 1:
                        pending_fused = (blk, emit_fused_mul(blk, gate, ctxt))
                    else:
                        # last block: fully inline (squares right after each
                        # sigmoid, finals on DVE) to minimize the kernel tail
                        fos = emit_fused_mul(blk, gate, ctxt)
                        emit_fused_norm(blk, fos)

            hold_cm.__exit__(None, None, None)

    nc.compile()
    return nc


_PROG_CACHE = {}


def _get_program(key, **kw):
    if key not in _PROG_CACHE:
        _PROG_CACHE[key] = build_program(**kw)
    return _PROG_CACHE[key]


def prepare(inputs):
    """Build (prog, in_maps) for the 8-core SPMD run."""
    return _prepare(inputs["hidden_states"], inputs["memory"], inputs["W_mem"],
                    inputs["W_gate"], inputs["b_gate"], inputs["norm_w"])


def _prepare(hidden_states, memory, W_mem, W_gate, b_gate, norm_w):
    B, N, D = hidden_states.shape
    _, M, E = memory.shape
    NC = 8
    H = NC // B                      # N-splits per batch (2)
    BN = N // H                      # rows per core (2048)

    prog = _get_program(("fp8", BN, M, D, E), BN=BN, M=M, D=D, E=E)

    import ml_dtypes
    f32 = np.float32
    bf16 = ml_dtypes.bfloat16
    fp8 = ml_dtypes.float8_e4m3
    WmT = np.ascontiguousarray(W_mem.T).astype(fp8)
    WghT = np.ascontiguousarray(W_gate[:, :D].T).astype(fp8)
    WgcT = np.ascontiguousarray(W_gate[:, D:].T).astype(fp8)
    bg = np.ascontiguousarray(b_gate[None, :]).astype(bf16)
    nw = np.ascontiguousarray(norm_w[None, :], dtype=f32)
    ident8 = np.eye(P, dtype=f32).astype(fp8)
    ident16 = np.eye(P, dtype=f32).astype(bf16)
    ones = np.ones((1, P), dtype=bf16)

    in_maps = []
    for c in range(NC):
        b, h = c // H, c % H
        hs = hidden_states[b, h * BN:(h + 1) * BN, :]
        in_maps.append({
            "hiddenT": np.ascontiguousarray(hs.T).astype(fp8),
            "hidden": np.ascontiguousarray(hs).astype(bf16),
            "memT": np.ascontiguousarray(memory[b].T).astype(fp8),
            "WmT": WmT, "WghT": WghT, "WgcT": WgcT,
            "b_gate": bg, "norm_w": nw,
            "ident8": ident8, "ident16": ident16, "ones": ones,
        })
    return prog, in_maps


def kernel(hidden_states, memory, W_mem, W_gate, b_gate, norm_w):
    from concourse.bass_utils import run_bass_kernel_spmd

    B, N, D = hidden_states.shape
    NC = 8
    H = NC // B
    BN = N // H
    prog, in_maps = _prepare(hidden_states, memory, W_mem, W_gate,
                             b_gate, norm_w)
    res = run_bass_kernel_spmd(prog, in_maps, core_ids=list(range(NC)))
    out = np.empty((B, N, D), dtype=np.float32)
    for c in range(NC):
        b, h = c // H, c % H
        out[b, h * BN:(h + 1) * BN, :] = res.results[c]["out"]
    return out
